# revision 38
# baseline (speedup 1.0000x reference)
"""Multi-head self-attention on 8 Trainium2 NeuronCores.

Sharding: core i handles batch b = i // 4 and head-group g = i % 4
(4 of 16 heads).  Tensor-parallel over heads for the QKV/attention/output
projection, data-parallel over batch.  Each core produces a partial
output (its head-group's slice of the final projection); the all-reduce
over the 4 head-group partials per batch happens on the host after the
gather, together with adding the output bias exactly once.

Device layout notes:
  - Host pre-transposes x to xT and pre-packs every tensor into the
    exact [128, F] SBUF image the kernel DMAs, so the device never
    transposes anything.
  - Everything on the PE runs bf16 (q/k/v/x/weights); exp(scores) is
    produced in bf16 both by the ACT engine (exact spline exp) and by
    the Vector engine via a Schraudolph integer approximation
    (tensor_scalar into int16 bits == bf16 exp), splitting the
    softmax-exp load across two engines.
  - Attention uses the scores-transposed layout: scoresT[k, q] tiles so
    that exp(scoresT) is directly the PV matmul's moving operand, and
    the softmax row sums come for free from a ones-column appended to
    the stationary V tile.  The two heads of a QK pair row-tile the PE
    array (contraction 64 each) and run concurrently.
  - Blocks run j-major; each block normalizes as soon as its PV ends
    (fp32 sums row -> broadcast DMA -> DVE approx reciprocal ->
    normalize multiplies on gpsimd/DVE), and the output projection for
    block j rides one block later inside the attention stream, so there
    is no serial post phase and the PE clock never re-throttles.
  - A memset-fed warm-up matmul burst right after the preamble opens
    the HAM clock gate before the first real projection arrives.
"""

import math
from collections import defaultdict

import numpy as np

B, S, D = 2, 2048, 1024
H, DH = 16, 64
NCORE = 8
TP = 4  # head-group shards per batch
HPC = H // TP  # heads per core
DHC = HPC * DH  # 256 = per-core slice of the model dim

_cache = {}

# Schraudolph exp in bf16: int16 bits = A*x + B with A folding the 1/8
# softmax scale; bf16 ULP makes round-vs-trunc immaterial.
SCH_A = 0.125 * (1 << 7) / math.log(2.0)
SCH_B = float(127 * (1 << 7)) - 5.6
# kt tiles handled by the Vector engine (rest go to ACT): 7 of 16
# (DVE's Schraudolph tile is ~1.2x slower than ACT's exp tile).
DVE_KT = frozenset((1, 3, 5, 7, 9, 11, 13))


def _build():
    import concourse.bacc as bacc
    import concourse.mybir as mybir
    import concourse.tile as tile

    F32 = mybir.dt.float32
    BF16 = mybir.dt.bfloat16
    I16 = mybir.dt.int16
    EXP = mybir.ActivationFunctionType.Exp
    MULT = mybir.AluOpType.mult
    ADD = mybir.AluOpType.add

    nc = bacc.Bacc("TRN2", target_bir_lowering=False, debug=False, num_devices=NCORE)

    # DRAM I/O (all images pre-packed on host)
    xt = nc.dram_tensor("xt", [4, 128, 4096], BF16, kind="ExternalInput").ap()
    wq = nc.dram_tensor("wq", [128, 2048], BF16, kind="ExternalInput").ap()
    wk = nc.dram_tensor("wk", [128, 2048], BF16, kind="ExternalInput").ap()
    wv = nc.dram_tensor("wv", [128, 2048], BF16, kind="ExternalInput").ap()
    wo = nc.dram_tensor("wo", [128, 2048], BF16, kind="ExternalInput").ap()
    bqk = nc.dram_tensor("bqk", [128, 4], F32, kind="ExternalInput").ap()
    bv = nc.dram_tensor("bv", [1, DHC], F32, kind="ExternalInput").ap()
    y = nc.dram_tensor("y", [S, D], BF16, kind="ExternalOutput").ap()

    with tile.TileContext(nc) as tc:
        with (
            tc.tile_pool(name="const", bufs=1) as pc,
            tc.tile_pool(name="w", bufs=1) as pw,
            tc.tile_pool(name="x", bufs=4) as px,
            tc.tile_pool(name="qkv", bufs=1) as pqkv,
            tc.tile_pool(name="pt", bufs=7) as ppt,
            tc.tile_pool(name="pti", bufs=7) as ppti,
            tc.tile_pool(name="er", bufs=1) as per,
            tc.tile_pool(name="r", bufs=2) as pr,
            tc.tile_pool(name="out", bufs=3) as pout,
            tc.tile_pool(name="mm", bufs=1, space="PSUM") as pmm,
            tc.tile_pool(name="pse", bufs=1, space="PSUM") as pse,
        ):
            # ---- weights/constants DMA in consumption order ----
            wq_sb = pw.tile([128, 2048], BF16, tag="wq")
            wk_sb = pw.tile([128, 2048], BF16, tag="wk")
            wv_sb = pw.tile([128, 2048], BF16, tag="wv")
            wo_sb = pw.tile([128, 2048], BF16, tag="wo")
            nc.scalar.dma_start(out=wk_sb[:], in_=wk[:])

            # ---- persistent activations ----
            qT = pqkv.tile([128, 4096], BF16, tag="qT")
            kT = pqkv.tile([128, 4096], BF16, tag="kT")
            # v (bf16): per seq-tile st: 4 heads x (64 v-cols + ones col)
            vsb = pqkv.tile([128, 16 * (DH + 1) * HPC], BF16, tag="v")
            ones_sb = pc.tile([128, 1], BF16, tag="ones")
            nc.vector.memset(ones_sb[:], 1.0)
            # ones row at partition 64: stationary operand of the rank-1
            # sums-broadcast matmul (matches srow's base partition)
            ones_row = pc.tile([65, 64], BF16, tag="ones_row")
            nc.vector.memset(ones_row[64:65, :], 1.0)
            nc.vector.tensor_copy(
                vsb[:].rearrange("p (st h c2) -> p st h c2", st=16, h=HPC)[
                    :, :, :, DH : DH + 1
                ],
                ones_sb[:].to_broadcast((128, 16, HPC, 1)),
            )
            embT = pqkv.tile([128, 4096], BF16, tag="embT")

            # ---- x DMA: chunk-major, pieces alternating sync/gpsimd queues;
            # weights flow on the scalar queue in consumption order so each
            # arrives just before its consumer (aggregate DMA is HBM-bound,
            # queue assignment controls ordering/fairness) ----
            xc_tiles = [
                px.tile([128, 4096], BF16, tag="xc", name=f"xc{c}") for c in range(4)
            ]
            bqk_sb = pc.tile([128, 4], F32)
            bv_sb = pc.tile([128, DHC], F32)
            nc.sync.dma_start(out=bqk_sb[:], in_=bqk[:])
            # chunks 0-1 (k-proj scp0's working set) ride all three queues
            # (scalar joins after wk); later chunks leave scalar for weights
            xq3 = (nc.sync, nc.gpsimd, nc.scalar)
            qi = 0
            for c in range(4):
                for o in range(0, 4096, 1024):
                    eng = xq3[qi % 3] if c < 2 else xq3[qi % 2]
                    qi += 1
                    eng.dma_start(
                        out=xc_tiles[c][:, o : o + 1024], in_=xt[c][:, o : o + 1024]
                    )
                if c == 1:
                    nc.scalar.dma_start(out=wv_sb[:], in_=wv[:])
                    nc.sync.dma_start(out=bv_sb[:], in_=bv.to_broadcast((128, DHC)))
                if c == 2:
                    nc.scalar.dma_start(out=wq_sb[:], in_=wq[:])
                if c == 3:
                    nc.scalar.dma_start(out=wo_sb[:], in_=wo[:])

            psn = [0]

            def mm_tile():
                psn[0] = (psn[0] + 1) % 3
                return pmm.tile(
                    [128, 1024], F32, tag=f"s{psn[0]}", name=f"ps{psn[0]}_{nc.next_id()}"
                )

            # ---- PE warm-up: 22 throwaway matmuls on a memset scratch tile
            # (no DMA dependency, so they start right after the preamble).
            # Keeps the HAM activity window busy so the clock gate opens
            # (1.2 -> 2.4 GHz) before the real projections start, bridging
            # the ~6us until wk + x chunk 0 have arrived.
            wu_sb = pc.tile([128, 512], BF16, tag="wu")
            nc.gpsimd.memset(wu_sb[:], 0.25)
            for _ in range(2):
                ps_wu = mm_tile()
                for r in range(12):
                    nc.tensor.matmul(
                        ps_wu[:, (r % 2) * 512 : (r % 2) * 512 + 512],
                        wu_sb[:, 0:128],
                        wu_sb[:],
                        start=True,
                        stop=True,
                    )

            def qk_proj(proj, wsb, tsb, boff, g2, scp, fine=False):
                # kt outer / chunk inner: both chunks' matmuls share the
                # same stationary weight slice, halving LDWEIGHTS traffic.
                # fine=True keeps chunk-outer order so the very first group
                # can start on chunk 0 before chunk 1 lands.
                chunks = (2 * scp, 2 * scp + 1)
                ps = mm_tile()
                loop = (
                    [(kt, ci) for ci in range(2) for kt in range(8)]
                    if fine
                    else [(kt, ci) for kt in range(8) for ci in range(2)]
                )
                for kt, ci in loop:
                    nc.tensor.matmul(
                        ps[:, ci * 512 : ci * 512 + 512],
                        wsb[:, kt * 256 + g2 * 128 : kt * 256 + g2 * 128 + 128],
                        xc_tiles[chunks[ci]][:, kt * 512 : kt * 512 + 512],
                        start=(kt == 0),
                        stop=(kt == 7),
                    )
                nc.scalar.add(
                    tsb[:, g2 * 2048 + scp * 1024 : g2 * 2048 + scp * 1024 + 1024],
                    ps[:],
                    bqk_sb[:, boff + g2 : boff + g2 + 1],
                )

            def v_proj(scp):
                for c in (2 * scp, 2 * scp + 1):
                    ps = mm_tile()
                    for stl in range(4):
                        for kt in range(8):
                            nc.tensor.matmul(
                                ps[:, stl * 256 : stl * 256 + 256],
                                xc_tiles[c][:, kt * 512 + stl * 128 : kt * 512 + stl * 128 + 128],
                                wv_sb[:, kt * 256 : kt * 256 + 256],
                                start=(kt == 0),
                                stop=(kt == 7),
                            )
                    for stl in range(4):
                        st = c * 4 + stl
                        vo = vsb[:, st * 260 : st * 260 + 260].rearrange(
                            "p (h c2) -> p h c2", h=HPC
                        )[:, :, 0:DH]
                        nc.vector.tensor_tensor(
                            out=vo,
                            in0=ps[:, stl * 256 : stl * 256 + 256].rearrange(
                                "p (h c2) -> p h c2", h=HPC
                            ),
                            in1=bv_sb[:].rearrange("p (h c2) -> p h c2", h=HPC),
                            op=ADD,
                        )

            # ---- pre-phase: K, V, Q projections (K first: attention dep) ----
            for scp in range(2):
                for g2 in range(2):
                    qk_proj("k", wk_sb, kT, 2, g2, scp, fine=(scp == 0 and g2 == 0))
                v_proj(scp)
                for g2 in range(2):
                    qk_proj("q", wq_sb, qT, 0, g2, scp)

            # ---- attention: one flattened (block, kt) stream, j-major ----
            # Raw PV outputs stage into er_all[64, 8K] bf16 (block blk
            # occupies cols [blk*1024, blk*1024+1024), hh halves side by
            # side).  Each block normalizes as soon as its PV finishes:
            # sums row -> fp32 srow (ACT) -> partition-broadcast via
            # SBUF->SBUF DMA -> approx reciprocal (DVE) -> normalize
            # multiplies (gpsimd for hh0, DVE shifted for hh1) into embT.
            # The j-major block order lets oproj(j) run one block later,
            # interleaved into the attention stream, so there is no serial
            # post phase and the PE never idles long enough to re-throttle.
            # QK+exp run LOOKAHEAD units ahead of PV in the in-order tensor
            # queue so the two exp engines always overlap, including across
            # block boundaries.
            er_all = per.tile([64, 8192], BF16, tag="er")
            pacc_map = {}
            exp_views = {}

            def emit_qk_exp(g2, j, kt):
                jo = g2 * 2048 + j * 512
                ko = g2 * 2048 + kt * 128
                ps = mm_tile()
                for hh in range(2):
                    plo = hh * 64
                    nc.tensor.matmul(
                        ps[:, hh * 512 : hh * 512 + 512],
                        kT[plo : plo + 64, ko : ko + 128],
                        qT[plo : plo + 64, jo : jo + 512],
                        start=True,
                        stop=True,
                    )
                if kt in DVE_KT:
                    pti = ppti.tile(
                        [128, 1024], I16, tag="pti", name=f"pti_{g2}_{j}_{kt}"
                    )
                    nc.vector.tensor_scalar(pti[:], ps[:], SCH_A, SCH_B, MULT, ADD)
                    exp_views[(g2, j, kt)] = pti[:].bitcast(BF16)
                else:
                    pt = ppt.tile(
                        [128, 1024], BF16, tag="pt", name=f"pt_{g2}_{j}_{kt}"
                    )
                    nc.scalar.activation(pt[:], ps[:], EXP, scale=0.125)
                    exp_views[(g2, j, kt)] = pt[:]

            def emit_pv(g2, j, kt):
                if kt == 0:
                    pacc_map[(g2, j)] = [
                        pse.tile([65, 512], F32, tag=f"pse{hh}", name=f"pse{hh}_{g2}_{j}")
                        for hh in range(2)
                    ]
                pacc = pacc_map[(g2, j)]
                ptm = exp_views.pop((g2, j, kt))
                for hh in range(2):
                    h = 2 * g2 + hh
                    nc.tensor.matmul(
                        pacc[hh][:],
                        vsb[:, kt * 260 + h * 65 : kt * 260 + h * 65 + 65],
                        ptm[:, hh * 512 : hh * 512 + 512],
                        start=(kt == 0),
                        stop=(kt == 15),
                    )
                if kt == 15:
                    blk = g2 * 4 + j
                    for hh in range(2):
                        dst = er_all[
                            0:64, blk * 1024 + hh * 512 : blk * 1024 + hh * 512 + 512
                        ]
                        if hh == 0:
                            nc.vector.tensor_copy(dst, pacc[hh][0:64, :])
                        else:
                            nc.scalar.copy(dst, pacc[hh][0:64, :])

            # Deferred emissions: the per-block normalize chain has multi-us
            # DMA latency inside it; emitting it all at once would park the
            # in-order DVE queue on the reciprocal while the broadcast DMA
            # flies.  Instead each stage is emitted a few units later, so
            # the engines always have exp work queued ahead of the waiters.
            pending = defaultdict(list)

            def norm_start(g2, j):
                blk = g2 * 4 + j
                pacc = pacc_map.pop((g2, j))
                # sums row out of PSUM (partition 64 stays partition 64).
                # The final block uses the vector engine: its exp queue
                # drains two units earlier than ACT's, so the tail's
                # normalize chain starts sooner and the PE stays warm.
                last = g2 == 1 and j == 3
                srow = pr.tile([65, 1024], BF16, tag=f"srow{blk % 2}", name=f"srow_{blk}")
                for hh in range(2):
                    dst = srow[64:65, hh * 512 : hh * 512 + 512]
                    if last:
                        nc.vector.tensor_copy(dst, pacc[hh][64:65, :])
                    else:
                        nc.scalar.copy(dst, pacc[hh][64:65, :])
                return srow

            def norm_bcast(srow):
                # rank-1 matmul broadcast: ones[1,64].T @ srow[1,1024] ->
                # [64,1024] fp32 in PSUM.  Runs on the tensor engine (cheap,
                # ~2x216ns) and avoids any DMA / gpsimd library churn in
                # the normalize chain.
                ps_rb = mm_tile()
                for hh in range(2):
                    nc.tensor.matmul(
                        ps_rb[0:64, hh * 512 : hh * 512 + 512],
                        ones_row[64:65, 0:64],
                        srow[64:65, hh * 512 : hh * 512 + 512],
                        start=True,
                        stop=True,
                    )
                return ps_rb

            def norm_recip(blk, ps_rb):
                rbi = pr.tile([64, 1024], F32, tag=f"rbi{blk % 2}", name=f"rbi_{blk}")
                nc.vector.reciprocal_approx_fast(out=rbi[:], in_=ps_rb[0:64, :])
                return rbi

            def norm_mul0(g2, j, rbi):
                blk = g2 * 4 + j
                jo = g2 * 2048 + j * 512
                nc.gpsimd.tensor_tensor(
                    out=embT[0:64, jo : jo + 512],
                    in0=er_all[0:64, blk * 1024 : blk * 1024 + 512],
                    in1=rbi[:, 0:512],
                    op=MULT,
                )

            def norm_mul1(g2, j, rbi):
                blk = g2 * 4 + j
                jo = g2 * 2048 + j * 512
                nc.vector.tensor_tensor(
                    out=embT[64:128, jo : jo + 512],
                    in0=er_all[0:64, blk * 1024 + 512 : blk * 1024 + 1024],
                    in1=rbi[:, 512:1024],
                    op=MULT,
                )

            def norm_block(g2, j, gi):
                srow = norm_start(g2, j)
                blk = g2 * 4 + j
                box = {}

                def stage_bcast():
                    box["ps_rb"] = norm_bcast(srow)

                def stage_recip():
                    box["rbi"] = norm_recip(blk, box["ps_rb"])

                # bcast waits only on the srow copies (ACT, a few us of
                # queue lag at most); recip follows one unit later so the
                # rotation tile it reads is released quickly
                pending[gi + 8].append(stage_bcast)
                pending[gi + 9].append(stage_recip)
                pending[gi + 11].append(lambda: norm_mul0(g2, j, box["rbi"]))
                pending[gi + 12].append(lambda: norm_mul1(g2, j, box["rbi"]))

            def oproj_qt(qt):
                # g2 outer / do inner: the embT stationary slice is shared
                # by both output halves
                ps_o = mm_tile()
                for g2 in range(2):
                    for do in range(2):
                        nc.tensor.matmul(
                            ps_o[:, do * 512 : do * 512 + 512],
                            embT[:, g2 * 2048 + qt * 128 : g2 * 2048 + qt * 128 + 128],
                            wo_sb[:, g2 * 1024 + do * 512 : g2 * 1024 + do * 512 + 512],
                            start=(g2 == 0),
                            stop=(g2 == 1),
                        )
                ot = pout.tile([128, 1024], BF16, tag="ot", name=f"ot{qt}")
                nc.vector.tensor_copy(ot[:, 0:512], ps_o[:, 0:512])
                if qt >= 12:
                    # tail groups: ship each half as soon as its copy lands
                    # so the final DMAs overlap the remaining drains
                    nc.sync.dma_start(
                        out=y[qt * 128 : qt * 128 + 128, 0:512], in_=ot[:, 0:512]
                    )
                    nc.scalar.copy(ot[:, 512:1024], ps_o[:, 512:1024])
                    nc.sync.dma_start(
                        out=y[qt * 128 : qt * 128 + 128, 512:1024], in_=ot[:, 512:1024]
                    )
                else:
                    # single whole-row DMA on the sync queue: the sync queue
                    # carries only y writes in steady state, so a late ot
                    # copy can never block another block's critical chain
                    nc.scalar.copy(ot[:, 512:1024], ps_o[:, 512:1024])
                    nc.sync.dma_start(
                        out=y[qt * 128 : qt * 128 + 128, :], in_=ot[:]
                    )

            def post_unit(gi, g2, j, kt):
                emit_pv(g2, j, kt)
                for fn in pending.pop(gi, ()):
                    fn()
                if kt == 15:
                    norm_block(g2, j, gi)
                # oproj(j-1) rides two blocks behind (after the deferred
                # normalize of block (1, j-1) lands at gi+19), spread wide;
                # the kt==15 slot doubles as boundary-stall cover while the
                # next block's pacc tiles wait on the er/srow drains
                if g2 == 1 and j >= 1 and kt in (4, 8, 12, 15):
                    oproj_qt(4 * (j - 1) + (4, 8, 12, 15).index(kt))

            LOOKAHEAD = 6
            seq = [
                (g2, j, kt) for j in range(4) for g2 in range(2) for kt in range(16)
            ]
            for gi, unit in enumerate(seq):
                emit_qk_exp(*unit)
                if gi >= LOOKAHEAD:
                    post_unit(gi - LOOKAHEAD, *seq[gi - LOOKAHEAD])
            for gi in range(len(seq) - LOOKAHEAD, len(seq)):
                post_unit(gi, *seq[gi])
            # tail: flush the last block's deferred normalize stages, then
            # the final output-projection group
            for gi in sorted(pending):
                for fn in pending[gi]:
                    fn()
            pending.clear()
            for qt in range(12, 16):
                oproj_qt(qt)

    nc.compile()
    return nc


def _pack_inputs(x, Wq, bq, Wk, bk, Wv, bv, Wo, bo):
    """Per-core host-side sharding into the exact DMA images (bf16)."""
    import ml_dtypes

    BF = ml_dtypes.bfloat16

    def img_w(Wslice):  # [1024, 256] -> [128, 8*256]
        return np.ascontiguousarray(
            Wslice.reshape(8, 128, DHC).transpose(1, 0, 2).reshape(128, 8 * DHC)
        ).astype(BF)

    in_maps = []
    for i in range(NCORE):
        b, g = i // TP, i % TP
        sl = slice(g * DHC, (g + 1) * DHC)
        xT = x[b].T  # [1024, 2048]
        xt_img = np.ascontiguousarray(
            xT.reshape(8, 128, 4, 512).transpose(2, 1, 0, 3).reshape(4, 128, 4096)
        ).astype(BF)
        bq_rs = bq[sl].reshape(2, 128).T  # [128, 2]
        bk_rs = bk[sl].reshape(2, 128).T
        bqk_img = np.ascontiguousarray(np.concatenate([bq_rs, bk_rs], axis=1))
        wo_img = np.ascontiguousarray(
            Wo[sl, :].reshape(2, 128, D).transpose(1, 0, 2).reshape(128, 2 * D)
        ).astype(BF)
        in_maps.append(
            {
                "xt": xt_img,
                "wq": img_w(Wq[:, sl]),
                "wk": img_w(Wk[:, sl]),
                "wv": img_w(Wv[:, sl]),
                "wo": wo_img,
                "bqk": bqk_img,
                "bv": np.ascontiguousarray(bv[sl].reshape(1, DHC)),
            }
        )
    return in_maps


def kernel(x, Wq, bq, Wk, bk, Wv, bv, Wo, bo, _trace=False):
    from concourse.bass_utils import run_bass_kernel_spmd

    args = [np.asarray(a, dtype=np.float32) for a in (x, Wq, bq, Wk, bk, Wv, bv, Wo, bo)]
    if "nc" not in _cache:
        _cache["nc"] = _build()
    nc = _cache["nc"]

    in_maps = _pack_inputs(*args)
    res = run_bass_kernel_spmd(nc, in_maps, list(range(NCORE)), trace=_trace)
    _cache["last_result"] = res

    out = np.zeros((B, S, D), dtype=np.float32)
    for i in range(NCORE):
        out[i // TP] += res.results[i]["y"].astype(np.float32)
    out += np.asarray(args[8])  # bo, added once per (b, s) row on the host
    return out



# revision 39
# speedup vs baseline: 1.2102x; 1.2102x over previous
"""Multi-head self-attention on 8 Trainium2 NeuronCores.

Sharding: core i handles batch b = i // 4 and head-group g = i % 4
(4 of 16 heads).  Tensor-parallel over heads for the QKV/attention/output
projection, data-parallel over batch.  Each core produces a partial
output (its head-group's slice of the final projection); the all-reduce
over the 4 head-group partials per batch happens on the host after the
gather, together with adding the output bias exactly once.

Device layout notes:
  - Host pre-transposes x to xT and pre-packs every tensor into the
    exact [128, F] SBUF image the kernel DMAs, so the device never
    transposes anything.
  - Everything on the PE runs bf16 (q/k/v/x/weights); exp(scores) is
    produced in bf16 both by the ACT engine (exact spline exp) and by
    the Vector engine via a Schraudolph integer approximation
    (tensor_scalar into int16 bits == bf16 exp), splitting the
    softmax-exp load across two engines.
  - Attention uses the scores-transposed layout: scoresT[k, q] tiles so
    that exp(scoresT) is directly the PV matmul's moving operand, and
    the softmax row sums come for free from a ones-column appended to
    the stationary V tile.  The two heads of a QK pair row-tile the PE
    array (contraction 64 each) and run concurrently.
  - Blocks run j-major; each block normalizes as soon as its PV ends
    (fp32 sums row -> broadcast DMA -> DVE approx reciprocal ->
    normalize multiplies on gpsimd/DVE), and the output projection for
    block j rides one block later inside the attention stream, so there
    is no serial post phase and the PE clock never re-throttles.
  - A memset-fed warm-up matmul burst right after the preamble opens
    the HAM clock gate before the first real projection arrives.
"""

import math
from collections import defaultdict

import numpy as np

B, S, D = 2, 2048, 1024
H, DH = 16, 64
NCORE = 8
TP = 4  # head-group shards per batch
HPC = H // TP  # heads per core
DHC = HPC * DH  # 256 = per-core slice of the model dim

_cache = {}

# Schraudolph exp in bf16: int16 bits = A*x + B with A folding the 1/8
# softmax scale; bf16 ULP makes round-vs-trunc immaterial.
SCH_A = 0.125 * (1 << 7) / math.log(2.0)
SCH_B = float(127 * (1 << 7)) - 5.6
# kt tiles handled by the Vector engine (rest go to ACT): 7 of 16
# (DVE's Schraudolph tile is ~1.2x slower than ACT's exp tile).
DVE_KT = frozenset((1, 3, 5, 7, 9, 11, 13))


def _build():
    import concourse.bacc as bacc
    import concourse.mybir as mybir
    import concourse.tile as tile

    F32 = mybir.dt.float32
    BF16 = mybir.dt.bfloat16
    I16 = mybir.dt.int16
    EXP = mybir.ActivationFunctionType.Exp
    MULT = mybir.AluOpType.mult
    ADD = mybir.AluOpType.add

    nc = bacc.Bacc("TRN2", target_bir_lowering=False, debug=False, num_devices=NCORE)

    # DRAM I/O (all images pre-packed on host)
    xt = nc.dram_tensor("xt", [4, 128, 4096], BF16, kind="ExternalInput").ap()
    wq = nc.dram_tensor("wq", [128, 2048], BF16, kind="ExternalInput").ap()
    wk = nc.dram_tensor("wk", [128, 2048], BF16, kind="ExternalInput").ap()
    wv = nc.dram_tensor("wv", [128, 2048], BF16, kind="ExternalInput").ap()
    wo = nc.dram_tensor("wo", [128, 2048], BF16, kind="ExternalInput").ap()
    bqk = nc.dram_tensor("bqk", [128, 4], F32, kind="ExternalInput").ap()
    bv = nc.dram_tensor("bv", [1, DHC], F32, kind="ExternalInput").ap()
    y = nc.dram_tensor("y", [S, D], BF16, kind="ExternalOutput").ap()

    with tile.TileContext(nc) as tc:
        with (
            tc.tile_pool(name="const", bufs=1) as pc,
            tc.tile_pool(name="w", bufs=1) as pw,
            tc.tile_pool(name="x", bufs=4) as px,
            tc.tile_pool(name="qkv", bufs=1) as pqkv,
            tc.tile_pool(name="pt", bufs=7) as ppt,
            tc.tile_pool(name="pti", bufs=7) as ppti,
            tc.tile_pool(name="er", bufs=1) as per,
            tc.tile_pool(name="r", bufs=2) as pr,
            tc.tile_pool(name="out", bufs=3) as pout,
            tc.tile_pool(name="mm", bufs=1, space="PSUM") as pmm,
            tc.tile_pool(name="pse", bufs=1, space="PSUM") as pse,
        ):
            # ---- weights/constants DMA in consumption order ----
            wq_sb = pw.tile([128, 2048], BF16, tag="wq")
            wk_sb = pw.tile([128, 2048], BF16, tag="wk")
            wv_sb = pw.tile([128, 2048], BF16, tag="wv")
            wo_sb = pw.tile([128, 2048], BF16, tag="wo")
            nc.scalar.dma_start(out=wk_sb[:], in_=wk[:])

            # ---- persistent activations ----
            qT = pqkv.tile([128, 4096], BF16, tag="qT")
            kT = pqkv.tile([128, 4096], BF16, tag="kT")
            # v (bf16): per seq-tile st: 4 heads x (64 v-cols + ones col)
            vsb = pqkv.tile([128, 16 * (DH + 1) * HPC], BF16, tag="v")
            ones_sb = pc.tile([128, 1], BF16, tag="ones")
            nc.vector.memset(ones_sb[:], 1.0)
            # ones row at partition 64: stationary operand of the rank-1
            # sums-broadcast matmul (matches srow's base partition)
            ones_row = pc.tile([65, 64], BF16, tag="ones_row")
            nc.vector.memset(ones_row[64:65, :], 1.0)
            nc.vector.tensor_copy(
                vsb[:].rearrange("p (st h c2) -> p st h c2", st=16, h=HPC)[
                    :, :, :, DH : DH + 1
                ],
                ones_sb[:].to_broadcast((128, 16, HPC, 1)),
            )
            embT = pqkv.tile([128, 4096], BF16, tag="embT")

            # ---- x DMA: chunk-major, pieces alternating sync/gpsimd queues;
            # weights flow on the scalar queue in consumption order so each
            # arrives just before its consumer (aggregate DMA is HBM-bound,
            # queue assignment controls ordering/fairness) ----
            xc_tiles = [
                px.tile([128, 4096], BF16, tag="xc", name=f"xc{c}") for c in range(4)
            ]
            bqk_sb = pc.tile([128, 4], F32)
            bv_sb = pc.tile([128, DHC], F32)
            nc.sync.dma_start(out=bqk_sb[:], in_=bqk[:])
            # chunks 0-1 (k-proj scp0's working set) ride all three queues
            # (scalar joins after wk); later chunks leave scalar for weights
            xq3 = (nc.sync, nc.gpsimd, nc.scalar)
            qi = 0
            for c in range(4):
                for o in range(0, 4096, 1024):
                    eng = xq3[qi % 3] if c < 2 else xq3[qi % 2]
                    qi += 1
                    eng.dma_start(
                        out=xc_tiles[c][:, o : o + 1024], in_=xt[c][:, o : o + 1024]
                    )
                if c == 1:
                    nc.scalar.dma_start(out=wv_sb[:], in_=wv[:])
                    nc.sync.dma_start(out=bv_sb[:], in_=bv.to_broadcast((128, DHC)))
                if c == 2:
                    nc.scalar.dma_start(out=wq_sb[:], in_=wq[:])
                if c == 3:
                    nc.scalar.dma_start(out=wo_sb[:], in_=wo[:])

            psn = [0]

            def mm_tile():
                psn[0] = (psn[0] + 1) % 3
                return pmm.tile(
                    [128, 1024], F32, tag=f"s{psn[0]}", name=f"ps{psn[0]}_{nc.next_id()}"
                )

            # ---- PE warm-up: 22 throwaway matmuls on a memset scratch tile
            # (no DMA dependency, so they start right after the preamble).
            # Keeps the HAM activity window busy so the clock gate opens
            # (1.2 -> 2.4 GHz) before the real projections start, bridging
            # the ~6us until wk + x chunk 0 have arrived.
            wu_sb = pc.tile([128, 512], BF16, tag="wu")
            nc.vector.memset(wu_sb[:], 0.25)
            for _ in range(2):
                ps_wu = mm_tile()
                for r in range(11):
                    nc.tensor.matmul(
                        ps_wu[:, (r % 2) * 512 : (r % 2) * 512 + 512],
                        wu_sb[:, 0:128],
                        wu_sb[:],
                        start=True,
                        stop=True,
                    )

            def qk_proj(proj, wsb, tsb, boff, g2, scp, fine=False):
                # kt outer / chunk inner: both chunks' matmuls share the
                # same stationary weight slice, halving LDWEIGHTS traffic.
                # fine=True keeps chunk-outer order so the very first group
                # can start on chunk 0 before chunk 1 lands.
                chunks = (2 * scp, 2 * scp + 1)
                ps = mm_tile()
                loop = (
                    [(kt, ci) for ci in range(2) for kt in range(8)]
                    if fine
                    else [(kt, ci) for kt in range(8) for ci in range(2)]
                )
                for kt, ci in loop:
                    nc.tensor.matmul(
                        ps[:, ci * 512 : ci * 512 + 512],
                        wsb[:, kt * 256 + g2 * 128 : kt * 256 + g2 * 128 + 128],
                        xc_tiles[chunks[ci]][:, kt * 512 : kt * 512 + 512],
                        start=(kt == 0),
                        stop=(kt == 7),
                    )
                nc.scalar.add(
                    tsb[:, g2 * 2048 + scp * 1024 : g2 * 2048 + scp * 1024 + 1024],
                    ps[:],
                    bqk_sb[:, boff + g2 : boff + g2 + 1],
                )

            def v_proj(scp):
                for c in (2 * scp, 2 * scp + 1):
                    ps = mm_tile()
                    for stl in range(4):
                        for kt in range(8):
                            nc.tensor.matmul(
                                ps[:, stl * 256 : stl * 256 + 256],
                                xc_tiles[c][:, kt * 512 + stl * 128 : kt * 512 + stl * 128 + 128],
                                wv_sb[:, kt * 256 : kt * 256 + 256],
                                start=(kt == 0),
                                stop=(kt == 7),
                            )
                    for stl in range(4):
                        st = c * 4 + stl
                        vo = vsb[:, st * 260 : st * 260 + 260].rearrange(
                            "p (h c2) -> p h c2", h=HPC
                        )[:, :, 0:DH]
                        nc.vector.tensor_tensor(
                            out=vo,
                            in0=ps[:, stl * 256 : stl * 256 + 256].rearrange(
                                "p (h c2) -> p h c2", h=HPC
                            ),
                            in1=bv_sb[:].rearrange("p (h c2) -> p h c2", h=HPC),
                            op=ADD,
                        )

            # ---- pre-phase: K, V, Q projections (K first: attention dep) ----
            for scp in range(2):
                for g2 in range(2):
                    qk_proj("k", wk_sb, kT, 2, g2, scp, fine=(scp == 0 and g2 == 0))
                v_proj(scp)
                for g2 in range(2):
                    qk_proj("q", wq_sb, qT, 0, g2, scp)

            # ---- attention: one flattened (block, kt) stream, j-major ----
            # Raw PV outputs stage into er_all[64, 8K] bf16 (block blk
            # occupies cols [blk*1024, blk*1024+1024), hh halves side by
            # side).  Each block normalizes as soon as its PV finishes:
            # sums row -> fp32 srow (ACT) -> partition-broadcast via
            # SBUF->SBUF DMA -> approx reciprocal (DVE) -> normalize
            # multiplies (gpsimd for hh0, DVE shifted for hh1) into embT.
            # The j-major block order lets oproj(j) run one block later,
            # interleaved into the attention stream, so there is no serial
            # post phase and the PE never idles long enough to re-throttle.
            # QK+exp run LOOKAHEAD units ahead of PV in the in-order tensor
            # queue so the two exp engines always overlap, including across
            # block boundaries.
            er_all = per.tile([64, 8192], BF16, tag="er")
            pacc_map = {}
            exp_views = {}

            def emit_qk_exp(g2, j, kt):
                jo = g2 * 2048 + j * 512
                ko = g2 * 2048 + kt * 128
                ps = mm_tile()
                for hh in range(2):
                    plo = hh * 64
                    nc.tensor.matmul(
                        ps[:, hh * 512 : hh * 512 + 512],
                        kT[plo : plo + 64, ko : ko + 128],
                        qT[plo : plo + 64, jo : jo + 512],
                        start=True,
                        stop=True,
                    )
                if kt in DVE_KT:
                    pti = ppti.tile(
                        [128, 1024], I16, tag="pti", name=f"pti_{g2}_{j}_{kt}"
                    )
                    nc.vector.tensor_scalar(pti[:], ps[:], SCH_A, SCH_B, MULT, ADD)
                    exp_views[(g2, j, kt)] = pti[:].bitcast(BF16)
                else:
                    pt = ppt.tile(
                        [128, 1024], BF16, tag="pt", name=f"pt_{g2}_{j}_{kt}"
                    )
                    nc.scalar.activation(pt[:], ps[:], EXP, scale=0.125)
                    exp_views[(g2, j, kt)] = pt[:]

            def emit_pv(g2, j, kt):
                if kt == 0:
                    pacc_map[(g2, j)] = [
                        pse.tile([65, 512], F32, tag=f"pse{hh}", name=f"pse{hh}_{g2}_{j}")
                        for hh in range(2)
                    ]
                pacc = pacc_map[(g2, j)]
                ptm = exp_views.pop((g2, j, kt))
                for hh in range(2):
                    h = 2 * g2 + hh
                    nc.tensor.matmul(
                        pacc[hh][:],
                        vsb[:, kt * 260 + h * 65 : kt * 260 + h * 65 + 65],
                        ptm[:, hh * 512 : hh * 512 + 512],
                        start=(kt == 0),
                        stop=(kt == 15),
                    )
                if kt == 15:
                    blk = g2 * 4 + j
                    for hh in range(2):
                        dst = er_all[
                            0:64, blk * 1024 + hh * 512 : blk * 1024 + hh * 512 + 512
                        ]
                        if hh == 0:
                            nc.vector.tensor_copy(dst, pacc[hh][0:64, :])
                        else:
                            nc.scalar.copy(dst, pacc[hh][0:64, :])

            # Deferred emissions: the per-block normalize chain has multi-us
            # DMA latency inside it; emitting it all at once would park the
            # in-order DVE queue on the reciprocal while the broadcast DMA
            # flies.  Instead each stage is emitted a few units later, so
            # the engines always have exp work queued ahead of the waiters.
            pending = defaultdict(list)

            def norm_start(g2, j):
                blk = g2 * 4 + j
                pacc = pacc_map.pop((g2, j))
                # sums row out of PSUM (partition 64 stays partition 64).
                # The final block uses the vector engine: its exp queue
                # drains two units earlier than ACT's, so the tail's
                # normalize chain starts sooner and the PE stays warm.
                last = g2 == 1 and j == 3
                srow = pr.tile([65, 1024], BF16, tag=f"srow{blk % 2}", name=f"srow_{blk}")
                for hh in range(2):
                    dst = srow[64:65, hh * 512 : hh * 512 + 512]
                    if last:
                        nc.vector.tensor_copy(dst, pacc[hh][64:65, :])
                    else:
                        nc.scalar.copy(dst, pacc[hh][64:65, :])
                return srow

            def norm_bcast(srow):
                # rank-1 matmul broadcast: ones[1,64].T @ srow[1,1024] ->
                # [64,1024] fp32 in PSUM.  Runs on the tensor engine (cheap,
                # ~2x216ns) and avoids any DMA / gpsimd library churn in
                # the normalize chain.
                ps_rb = mm_tile()
                for hh in range(2):
                    nc.tensor.matmul(
                        ps_rb[0:64, hh * 512 : hh * 512 + 512],
                        ones_row[64:65, 0:64],
                        srow[64:65, hh * 512 : hh * 512 + 512],
                        start=True,
                        stop=True,
                    )
                return ps_rb

            def norm_recip(blk, ps_rb):
                rbi = pr.tile([64, 1024], F32, tag=f"rbi{blk % 2}", name=f"rbi_{blk}")
                nc.vector.reciprocal_approx_fast(out=rbi[:], in_=ps_rb[0:64, :])
                return rbi

            def norm_mul0(g2, j, rbi):
                blk = g2 * 4 + j
                jo = g2 * 2048 + j * 512
                nc.gpsimd.tensor_tensor(
                    out=embT[0:64, jo : jo + 512],
                    in0=er_all[0:64, blk * 1024 : blk * 1024 + 512],
                    in1=rbi[:, 0:512],
                    op=MULT,
                )

            def norm_mul1(g2, j, rbi):
                blk = g2 * 4 + j
                jo = g2 * 2048 + j * 512
                nc.vector.tensor_tensor(
                    out=embT[64:128, jo : jo + 512],
                    in0=er_all[0:64, blk * 1024 + 512 : blk * 1024 + 1024],
                    in1=rbi[:, 512:1024],
                    op=MULT,
                )

            def norm_block(g2, j, gi):
                srow = norm_start(g2, j)
                blk = g2 * 4 + j
                box = {}

                def stage_bcast():
                    box["ps_rb"] = norm_bcast(srow)

                def stage_recip():
                    box["rbi"] = norm_recip(blk, box["ps_rb"])

                # bcast waits only on the srow copies (ACT, a few us of
                # queue lag at most); recip follows one unit later so the
                # rotation tile it reads is released quickly
                pending[gi + 8].append(stage_bcast)
                pending[gi + 9].append(stage_recip)
                pending[gi + 11].append(lambda: norm_mul0(g2, j, box["rbi"]))
                pending[gi + 12].append(lambda: norm_mul1(g2, j, box["rbi"]))

            def oproj_qt(qt):
                # g2 outer / do inner: the embT stationary slice is shared
                # by both output halves
                ps_o = mm_tile()
                for g2 in range(2):
                    for do in range(2):
                        nc.tensor.matmul(
                            ps_o[:, do * 512 : do * 512 + 512],
                            embT[:, g2 * 2048 + qt * 128 : g2 * 2048 + qt * 128 + 128],
                            wo_sb[:, g2 * 1024 + do * 512 : g2 * 1024 + do * 512 + 512],
                            start=(g2 == 0),
                            stop=(g2 == 1),
                        )
                ot = pout.tile([128, 1024], BF16, tag="ot", name=f"ot{qt}")
                nc.vector.tensor_copy(ot[:, 0:512], ps_o[:, 0:512])
                if qt >= 12:
                    # tail groups: ship each half as soon as its copy lands
                    # so the final DMAs overlap the remaining drains
                    nc.sync.dma_start(
                        out=y[qt * 128 : qt * 128 + 128, 0:512], in_=ot[:, 0:512]
                    )
                    nc.scalar.copy(ot[:, 512:1024], ps_o[:, 512:1024])
                    nc.sync.dma_start(
                        out=y[qt * 128 : qt * 128 + 128, 512:1024], in_=ot[:, 512:1024]
                    )
                else:
                    # single whole-row DMA on the sync queue: the sync queue
                    # carries only y writes in steady state, so a late ot
                    # copy can never block another block's critical chain
                    nc.scalar.copy(ot[:, 512:1024], ps_o[:, 512:1024])
                    nc.sync.dma_start(
                        out=y[qt * 128 : qt * 128 + 128, :], in_=ot[:]
                    )

            def post_unit(gi, g2, j, kt):
                emit_pv(g2, j, kt)
                for fn in pending.pop(gi, ()):
                    fn()
                if kt == 15:
                    norm_block(g2, j, gi)
                # oproj(j-1) rides two blocks behind (after the deferred
                # normalize of block (1, j-1) lands at gi+19), spread wide;
                # the kt==15 slot doubles as boundary-stall cover while the
                # next block's pacc tiles wait on the er/srow drains
                if g2 == 1 and j >= 1 and kt in (4, 8, 12, 15):
                    oproj_qt(4 * (j - 1) + (4, 8, 12, 15).index(kt))

            LOOKAHEAD = 6
            seq = [
                (g2, j, kt) for j in range(4) for g2 in range(2) for kt in range(16)
            ]
            for gi, unit in enumerate(seq):
                emit_qk_exp(*unit)
                if gi >= LOOKAHEAD:
                    post_unit(gi - LOOKAHEAD, *seq[gi - LOOKAHEAD])
            for gi in range(len(seq) - LOOKAHEAD, len(seq)):
                post_unit(gi, *seq[gi])
            # tail: flush the last block's deferred normalize stages, then
            # the final output-projection group
            for gi in sorted(pending):
                for fn in pending[gi]:
                    fn()
            pending.clear()
            for qt in range(12, 16):
                oproj_qt(qt)

    nc.compile()
    return nc


def _pack_inputs(x, Wq, bq, Wk, bk, Wv, bv, Wo, bo):
    """Per-core host-side sharding into the exact DMA images (bf16)."""
    import ml_dtypes

    BF = ml_dtypes.bfloat16

    def img_w(Wslice):  # [1024, 256] -> [128, 8*256]
        return np.ascontiguousarray(
            Wslice.reshape(8, 128, DHC).transpose(1, 0, 2).reshape(128, 8 * DHC)
        ).astype(BF)

    in_maps = []
    for i in range(NCORE):
        b, g = i // TP, i % TP
        sl = slice(g * DHC, (g + 1) * DHC)
        xT = x[b].T  # [1024, 2048]
        xt_img = np.ascontiguousarray(
            xT.reshape(8, 128, 4, 512).transpose(2, 1, 0, 3).reshape(4, 128, 4096)
        ).astype(BF)
        bq_rs = bq[sl].reshape(2, 128).T  # [128, 2]
        bk_rs = bk[sl].reshape(2, 128).T
        bqk_img = np.ascontiguousarray(np.concatenate([bq_rs, bk_rs], axis=1))
        wo_img = np.ascontiguousarray(
            Wo[sl, :].reshape(2, 128, D).transpose(1, 0, 2).reshape(128, 2 * D)
        ).astype(BF)
        in_maps.append(
            {
                "xt": xt_img,
                "wq": img_w(Wq[:, sl]),
                "wk": img_w(Wk[:, sl]),
                "wv": img_w(Wv[:, sl]),
                "wo": wo_img,
                "bqk": bqk_img,
                "bv": np.ascontiguousarray(bv[sl].reshape(1, DHC)),
            }
        )
    return in_maps


def kernel(x, Wq, bq, Wk, bk, Wv, bv, Wo, bo, _trace=False):
    from concourse.bass_utils import run_bass_kernel_spmd

    args = [np.asarray(a, dtype=np.float32) for a in (x, Wq, bq, Wk, bk, Wv, bv, Wo, bo)]
    if "nc" not in _cache:
        _cache["nc"] = _build()
    nc = _cache["nc"]

    in_maps = _pack_inputs(*args)
    res = run_bass_kernel_spmd(nc, in_maps, list(range(NCORE)), trace=_trace)
    _cache["last_result"] = res

    out = np.zeros((B, S, D), dtype=np.float32)
    for i in range(NCORE):
        out[i // TP] += res.results[i]["y"].astype(np.float32)
    out += np.asarray(args[8])  # bo, added once per (b, s) row on the host
    return out



# revision 40
# speedup vs baseline: 1.2151x; 1.0040x over previous
"""Multi-head self-attention on 8 Trainium2 NeuronCores.

Sharding: core i handles batch b = i // 4 and head-group g = i % 4
(4 of 16 heads).  Tensor-parallel over heads for the QKV/attention/output
projection, data-parallel over batch.  Each core produces a partial
output (its head-group's slice of the final projection); the all-reduce
over the 4 head-group partials per batch happens on the host after the
gather, together with adding the output bias exactly once.

Device layout notes:
  - Host pre-transposes x to xT and pre-packs every tensor into the
    exact [128, F] SBUF image the kernel DMAs, so the device never
    transposes anything.
  - Everything on the PE runs bf16 (q/k/v/x/weights); exp(scores) is
    produced in bf16 both by the ACT engine (exact spline exp) and by
    the Vector engine via a Schraudolph integer approximation
    (tensor_scalar into int16 bits == bf16 exp), splitting the
    softmax-exp load across two engines.
  - Attention uses the scores-transposed layout: scoresT[k, q] tiles so
    that exp(scoresT) is directly the PV matmul's moving operand, and
    the softmax row sums come for free from a ones-column appended to
    the stationary V tile.  The two heads of a QK pair row-tile the PE
    array (contraction 64 each) and run concurrently.
  - Blocks run j-major; each block normalizes as soon as its PV ends
    (fp32 sums row -> broadcast DMA -> DVE approx reciprocal ->
    normalize multiplies on gpsimd/DVE), and the output projection for
    block j rides one block later inside the attention stream, so there
    is no serial post phase and the PE clock never re-throttles.
  - A memset-fed warm-up matmul burst right after the preamble opens
    the HAM clock gate before the first real projection arrives.
"""

import math
from collections import defaultdict

import numpy as np

B, S, D = 2, 2048, 1024
H, DH = 16, 64
NCORE = 8
TP = 4  # head-group shards per batch
HPC = H // TP  # heads per core
DHC = HPC * DH  # 256 = per-core slice of the model dim

_cache = {}

# Schraudolph exp in bf16: int16 bits = A*x + B with A folding the 1/8
# softmax scale; bf16 ULP makes round-vs-trunc immaterial.
SCH_A = 0.125 * (1 << 7) / math.log(2.0)
SCH_B = float(127 * (1 << 7)) - 5.6
# kt tiles handled by the Vector engine (rest go to ACT): 7 of 16
# (DVE's Schraudolph tile is ~1.2x slower than ACT's exp tile).
DVE_KT = frozenset((1, 3, 5, 7, 9, 11, 13))


def _build():
    import concourse.bacc as bacc
    import concourse.mybir as mybir
    import concourse.tile as tile

    F32 = mybir.dt.float32
    BF16 = mybir.dt.bfloat16
    I16 = mybir.dt.int16
    EXP = mybir.ActivationFunctionType.Exp
    MULT = mybir.AluOpType.mult
    ADD = mybir.AluOpType.add

    nc = bacc.Bacc("TRN2", target_bir_lowering=False, debug=False, num_devices=NCORE)

    # DRAM I/O (all images pre-packed on host)
    xt = nc.dram_tensor("xt", [4, 128, 4096], BF16, kind="ExternalInput").ap()
    wq = nc.dram_tensor("wq", [128, 2048], BF16, kind="ExternalInput").ap()
    wk = nc.dram_tensor("wk", [128, 2048], BF16, kind="ExternalInput").ap()
    wv = nc.dram_tensor("wv", [128, 2048], BF16, kind="ExternalInput").ap()
    wo = nc.dram_tensor("wo", [128, 2048], BF16, kind="ExternalInput").ap()
    bqk = nc.dram_tensor("bqk", [128, 4], F32, kind="ExternalInput").ap()
    bv = nc.dram_tensor("bv", [1, DHC], F32, kind="ExternalInput").ap()
    y = nc.dram_tensor("y", [S, D], BF16, kind="ExternalOutput").ap()

    with tile.TileContext(nc) as tc:
        with (
            tc.tile_pool(name="const", bufs=1) as pc,
            tc.tile_pool(name="w", bufs=1) as pw,
            tc.tile_pool(name="x", bufs=4) as px,
            tc.tile_pool(name="qkv", bufs=1) as pqkv,
            tc.tile_pool(name="pt", bufs=7) as ppt,
            tc.tile_pool(name="pti", bufs=7) as ppti,
            tc.tile_pool(name="er", bufs=1) as per,
            tc.tile_pool(name="r", bufs=2) as pr,
            tc.tile_pool(name="out", bufs=3) as pout,
            tc.tile_pool(name="mm", bufs=1, space="PSUM") as pmm,
            tc.tile_pool(name="pse", bufs=1, space="PSUM") as pse,
        ):
            # ---- weights/constants DMA in consumption order ----
            wq_sb = pw.tile([128, 2048], BF16, tag="wq")
            wk_sb = pw.tile([128, 2048], BF16, tag="wk")
            wv_sb = pw.tile([128, 2048], BF16, tag="wv")
            wo_sb = pw.tile([128, 2048], BF16, tag="wo")
            nc.scalar.dma_start(out=wk_sb[:], in_=wk[:])

            # ---- persistent activations ----
            qT = pqkv.tile([128, 4096], BF16, tag="qT")
            kT = pqkv.tile([128, 4096], BF16, tag="kT")
            # v (bf16): per seq-tile st: 4 heads x (64 v-cols + ones col)
            vsb = pqkv.tile([128, 16 * (DH + 1) * HPC], BF16, tag="v")
            ones_sb = pc.tile([128, 1], BF16, tag="ones")
            nc.vector.memset(ones_sb[:], 1.0)
            # ones row at partition 64: stationary operand of the rank-1
            # sums-broadcast matmul (matches srow's base partition)
            ones_row = pc.tile([65, 64], BF16, tag="ones_row")
            nc.vector.memset(ones_row[64:65, :], 1.0)
            nc.vector.tensor_copy(
                vsb[:].rearrange("p (st h c2) -> p st h c2", st=16, h=HPC)[
                    :, :, :, DH : DH + 1
                ],
                ones_sb[:].to_broadcast((128, 16, HPC, 1)),
            )
            embT = pqkv.tile([128, 4096], BF16, tag="embT")

            # ---- x DMA: chunk-major, pieces alternating sync/gpsimd queues;
            # weights flow on the scalar queue in consumption order so each
            # arrives just before its consumer (aggregate DMA is HBM-bound,
            # queue assignment controls ordering/fairness) ----
            xc_tiles = [
                px.tile([128, 4096], BF16, tag="xc", name=f"xc{c}") for c in range(4)
            ]
            bqk_sb = pc.tile([128, 4], F32)
            bv_sb = pc.tile([128, DHC], F32)
            nc.sync.dma_start(out=bqk_sb[:], in_=bqk[:])
            # chunks 0-1 (k-proj scp0's working set) ride all three queues
            # (scalar joins after wk); later chunks leave scalar for weights
            xq3 = (nc.sync, nc.gpsimd, nc.scalar)
            qi = 0
            for c in range(4):
                for o in range(0, 4096, 1024):
                    eng = xq3[qi % 3] if c < 2 else xq3[qi % 2]
                    qi += 1
                    eng.dma_start(
                        out=xc_tiles[c][:, o : o + 1024], in_=xt[c][:, o : o + 1024]
                    )
                if c == 1:
                    nc.scalar.dma_start(out=wv_sb[:], in_=wv[:])
                    nc.sync.dma_start(out=bv_sb[:], in_=bv.to_broadcast((128, DHC)))
                if c == 2:
                    nc.scalar.dma_start(out=wq_sb[:], in_=wq[:])
                if c == 3:
                    nc.scalar.dma_start(out=wo_sb[:], in_=wo[:])

            psn = [0]

            def mm_tile():
                psn[0] = (psn[0] + 1) % 3
                return pmm.tile(
                    [128, 1024], F32, tag=f"s{psn[0]}", name=f"ps{psn[0]}_{nc.next_id()}"
                )

            # ---- PE warm-up: 22 throwaway matmuls on a memset scratch tile
            # (no DMA dependency, so they start right after the preamble).
            # Keeps the HAM activity window busy so the clock gate opens
            # (1.2 -> 2.4 GHz) before the real projections start, bridging
            # the ~6us until wk + x chunk 0 have arrived.
            wu_sb = pc.tile([128, 512], BF16, tag="wu")
            nc.vector.memset(wu_sb[:], 0.25)
            for _ in range(2):
                ps_wu = mm_tile()
                for r in range(11):
                    nc.tensor.matmul(
                        ps_wu[:, (r % 2) * 512 : (r % 2) * 512 + 512],
                        wu_sb[:, 0:128],
                        wu_sb[:],
                        start=True,
                        stop=True,
                    )

            def qk_proj(proj, wsb, tsb, boff, g2, scp, fine=False):
                # kt outer / chunk inner: both chunks' matmuls share the
                # same stationary weight slice, halving LDWEIGHTS traffic.
                # fine=True keeps chunk-outer order so the very first group
                # can start on chunk 0 before chunk 1 lands.
                chunks = (2 * scp, 2 * scp + 1)
                ps = mm_tile()
                loop = (
                    [(kt, ci) for ci in range(2) for kt in range(8)]
                    if fine
                    else [(kt, ci) for kt in range(8) for ci in range(2)]
                )
                for kt, ci in loop:
                    nc.tensor.matmul(
                        ps[:, ci * 512 : ci * 512 + 512],
                        wsb[:, kt * 256 + g2 * 128 : kt * 256 + g2 * 128 + 128],
                        xc_tiles[chunks[ci]][:, kt * 512 : kt * 512 + 512],
                        start=(kt == 0),
                        stop=(kt == 7),
                    )
                nc.scalar.add(
                    tsb[:, g2 * 2048 + scp * 1024 : g2 * 2048 + scp * 1024 + 1024],
                    ps[:],
                    bqk_sb[:, boff + g2 : boff + g2 + 1],
                )

            def v_proj(scp):
                for c in (2 * scp, 2 * scp + 1):
                    ps = mm_tile()
                    for stl in range(4):
                        for kt in range(8):
                            nc.tensor.matmul(
                                ps[:, stl * 256 : stl * 256 + 256],
                                xc_tiles[c][:, kt * 512 + stl * 128 : kt * 512 + stl * 128 + 128],
                                wv_sb[:, kt * 256 : kt * 256 + 256],
                                start=(kt == 0),
                                stop=(kt == 7),
                            )
                    for stl in range(4):
                        st = c * 4 + stl
                        vo = vsb[:, st * 260 : st * 260 + 260].rearrange(
                            "p (h c2) -> p h c2", h=HPC
                        )[:, :, 0:DH]
                        nc.vector.tensor_tensor(
                            out=vo,
                            in0=ps[:, stl * 256 : stl * 256 + 256].rearrange(
                                "p (h c2) -> p h c2", h=HPC
                            ),
                            in1=bv_sb[:].rearrange("p (h c2) -> p h c2", h=HPC),
                            op=ADD,
                        )

            # ---- pre-phase: K, V, Q projections (K first: attention dep) ----
            for scp in range(2):
                for g2 in range(2):
                    qk_proj("k", wk_sb, kT, 2, g2, scp, fine=(scp == 0 and g2 == 0))
                v_proj(scp)
                for g2 in range(2):
                    qk_proj("q", wq_sb, qT, 0, g2, scp)

            # ---- attention: one flattened (block, kt) stream, j-major ----
            # Raw PV outputs stage into er_all[64, 8K] bf16 (block blk
            # occupies cols [blk*1024, blk*1024+1024), hh halves side by
            # side).  Each block normalizes as soon as its PV finishes:
            # sums row -> fp32 srow (ACT) -> partition-broadcast via
            # SBUF->SBUF DMA -> approx reciprocal (DVE) -> normalize
            # multiplies (gpsimd for hh0, DVE shifted for hh1) into embT.
            # The j-major block order lets oproj(j) run one block later,
            # interleaved into the attention stream, so there is no serial
            # post phase and the PE never idles long enough to re-throttle.
            # QK+exp run LOOKAHEAD units ahead of PV in the in-order tensor
            # queue so the two exp engines always overlap, including across
            # block boundaries.
            er_all = per.tile([64, 8192], BF16, tag="er")
            pacc_map = {}
            exp_views = {}

            def emit_qk_exp(g2, j, kt):
                jo = g2 * 2048 + j * 512
                ko = g2 * 2048 + kt * 128
                ps = mm_tile()
                for hh in range(2):
                    plo = hh * 64
                    nc.tensor.matmul(
                        ps[:, hh * 512 : hh * 512 + 512],
                        kT[plo : plo + 64, ko : ko + 128],
                        qT[plo : plo + 64, jo : jo + 512],
                        start=True,
                        stop=True,
                    )
                if kt in DVE_KT:
                    pti = ppti.tile(
                        [128, 1024], I16, tag="pti", name=f"pti_{g2}_{j}_{kt}"
                    )
                    nc.vector.tensor_scalar(pti[:], ps[:], SCH_A, SCH_B, MULT, ADD)
                    exp_views[(g2, j, kt)] = pti[:].bitcast(BF16)
                else:
                    pt = ppt.tile(
                        [128, 1024], BF16, tag="pt", name=f"pt_{g2}_{j}_{kt}"
                    )
                    nc.scalar.activation(pt[:], ps[:], EXP, scale=0.125)
                    exp_views[(g2, j, kt)] = pt[:]

            def emit_pv(g2, j, kt):
                if kt == 0:
                    pacc_map[(g2, j)] = [
                        pse.tile([65, 512], F32, tag=f"pse{hh}", name=f"pse{hh}_{g2}_{j}")
                        for hh in range(2)
                    ]
                pacc = pacc_map[(g2, j)]
                ptm = exp_views.pop((g2, j, kt))
                for hh in range(2):
                    h = 2 * g2 + hh
                    nc.tensor.matmul(
                        pacc[hh][:],
                        vsb[:, kt * 260 + h * 65 : kt * 260 + h * 65 + 65],
                        ptm[:, hh * 512 : hh * 512 + 512],
                        start=(kt == 0),
                        stop=(kt == 15),
                    )
                if kt == 15:
                    blk = g2 * 4 + j
                    for hh in range(2):
                        dst = er_all[
                            0:64, blk * 1024 + hh * 512 : blk * 1024 + hh * 512 + 512
                        ]
                        if hh == 0:
                            nc.vector.tensor_copy(dst, pacc[hh][0:64, :])
                        else:
                            nc.scalar.copy(dst, pacc[hh][0:64, :])

            # Deferred emissions: the per-block normalize chain has multi-us
            # DMA latency inside it; emitting it all at once would park the
            # in-order DVE queue on the reciprocal while the broadcast DMA
            # flies.  Instead each stage is emitted a few units later, so
            # the engines always have exp work queued ahead of the waiters.
            pending = defaultdict(list)

            def norm_start(g2, j):
                blk = g2 * 4 + j
                pacc = pacc_map.pop((g2, j))
                # sums row out of PSUM (partition 64 stays partition 64).
                # The final block uses the vector engine: its exp queue
                # drains two units earlier than ACT's, so the tail's
                # normalize chain starts sooner and the PE stays warm.
                last = g2 == 1 and j == 3
                srow = pr.tile([65, 1024], BF16, tag=f"srow{blk % 2}", name=f"srow_{blk}")
                for hh in range(2):
                    dst = srow[64:65, hh * 512 : hh * 512 + 512]
                    if last:
                        nc.vector.tensor_copy(dst, pacc[hh][64:65, :])
                    else:
                        nc.scalar.copy(dst, pacc[hh][64:65, :])
                return srow

            def norm_bcast(srow):
                # rank-1 matmul broadcast: ones[1,64].T @ srow[1,1024] ->
                # [64,1024] fp32 in PSUM.  Runs on the tensor engine (cheap,
                # ~2x216ns) and avoids any DMA / gpsimd library churn in
                # the normalize chain.
                ps_rb = mm_tile()
                for hh in range(2):
                    nc.tensor.matmul(
                        ps_rb[0:64, hh * 512 : hh * 512 + 512],
                        ones_row[64:65, 0:64],
                        srow[64:65, hh * 512 : hh * 512 + 512],
                        start=True,
                        stop=True,
                    )
                return ps_rb

            def norm_recip(blk, ps_rb):
                rbi = pr.tile([64, 1024], F32, tag=f"rbi{blk % 2}", name=f"rbi_{blk}")
                nc.vector.reciprocal_approx_fast(out=rbi[:], in_=ps_rb[0:64, :])
                return rbi

            def norm_mul0(g2, j, rbi):
                blk = g2 * 4 + j
                jo = g2 * 2048 + j * 512
                nc.gpsimd.tensor_tensor(
                    out=embT[0:64, jo : jo + 512],
                    in0=er_all[0:64, blk * 1024 : blk * 1024 + 512],
                    in1=rbi[:, 0:512],
                    op=MULT,
                )

            def norm_mul1(g2, j, rbi):
                blk = g2 * 4 + j
                jo = g2 * 2048 + j * 512
                nc.vector.tensor_tensor(
                    out=embT[64:128, jo : jo + 512],
                    in0=er_all[0:64, blk * 1024 + 512 : blk * 1024 + 1024],
                    in1=rbi[:, 512:1024],
                    op=MULT,
                )

            def norm_block(g2, j, gi):
                srow = norm_start(g2, j)
                blk = g2 * 4 + j
                box = {}

                def stage_bcast():
                    box["ps_rb"] = norm_bcast(srow)

                def stage_recip():
                    box["rbi"] = norm_recip(blk, box["ps_rb"])

                # bcast waits only on the srow copies (ACT, a few us of
                # queue lag at most); recip follows one unit later so the
                # rotation tile it reads is released quickly
                pending[gi + 8].append(stage_bcast)
                pending[gi + 9].append(stage_recip)
                pending[gi + 11].append(lambda: norm_mul0(g2, j, box["rbi"]))
                pending[gi + 12].append(lambda: norm_mul1(g2, j, box["rbi"]))

            def oproj_qt(qt):
                # g2 outer / do inner: the embT stationary slice is shared
                # by both output halves
                ps_o = mm_tile()
                for g2 in range(2):
                    for do in range(2):
                        nc.tensor.matmul(
                            ps_o[:, do * 512 : do * 512 + 512],
                            embT[:, g2 * 2048 + qt * 128 : g2 * 2048 + qt * 128 + 128],
                            wo_sb[:, g2 * 1024 + do * 512 : g2 * 1024 + do * 512 + 512],
                            start=(g2 == 0),
                            stop=(g2 == 1),
                        )
                ot = pout.tile([128, 1024], BF16, tag="ot", name=f"ot{qt}")
                nc.vector.tensor_copy(ot[:, 0:512], ps_o[:, 0:512])
                nc.scalar.copy(ot[:, 512:1024], ps_o[:, 512:1024])
                # single whole-row DMA on the sync queue: the sync queue
                # carries only y writes in steady state, so a late ot copy
                # can never block another block's critical chain
                nc.sync.dma_start(
                    out=y[qt * 128 : qt * 128 + 128, :], in_=ot[:]
                )

            def post_unit(gi, g2, j, kt):
                emit_pv(g2, j, kt)
                for fn in pending.pop(gi, ()):
                    fn()
                if kt == 15:
                    norm_block(g2, j, gi)
                # oproj(j-1) rides two blocks behind (after the deferred
                # normalize of block (1, j-1) lands at gi+19), spread wide;
                # the kt==15 slot doubles as boundary-stall cover while the
                # next block's pacc tiles wait on the er/srow drains
                if g2 == 1 and j >= 1 and kt in (4, 8, 12, 15):
                    oproj_qt(4 * (j - 1) + (4, 8, 12, 15).index(kt))

            LOOKAHEAD = 6
            seq = [
                (g2, j, kt) for j in range(4) for g2 in range(2) for kt in range(16)
            ]
            for gi, unit in enumerate(seq):
                emit_qk_exp(*unit)
                if gi >= LOOKAHEAD:
                    post_unit(gi - LOOKAHEAD, *seq[gi - LOOKAHEAD])
            for gi in range(len(seq) - LOOKAHEAD, len(seq)):
                post_unit(gi, *seq[gi])
            # tail: flush the last block's deferred normalize stages, then
            # the final output-projection group
            for gi in sorted(pending):
                for fn in pending[gi]:
                    fn()
            pending.clear()
            for qt in range(12, 16):
                oproj_qt(qt)

    nc.compile()
    return nc


def _pack_inputs(x, Wq, bq, Wk, bk, Wv, bv, Wo, bo):
    """Per-core host-side sharding into the exact DMA images (bf16)."""
    import ml_dtypes

    BF = ml_dtypes.bfloat16

    def img_w(Wslice):  # [1024, 256] -> [128, 8*256]
        return np.ascontiguousarray(
            Wslice.reshape(8, 128, DHC).transpose(1, 0, 2).reshape(128, 8 * DHC)
        ).astype(BF)

    in_maps = []
    for i in range(NCORE):
        b, g = i // TP, i % TP
        sl = slice(g * DHC, (g + 1) * DHC)
        xT = x[b].T  # [1024, 2048]
        xt_img = np.ascontiguousarray(
            xT.reshape(8, 128, 4, 512).transpose(2, 1, 0, 3).reshape(4, 128, 4096)
        ).astype(BF)
        bq_rs = bq[sl].reshape(2, 128).T  # [128, 2]
        bk_rs = bk[sl].reshape(2, 128).T
        bqk_img = np.ascontiguousarray(np.concatenate([bq_rs, bk_rs], axis=1))
        wo_img = np.ascontiguousarray(
            Wo[sl, :].reshape(2, 128, D).transpose(1, 0, 2).reshape(128, 2 * D)
        ).astype(BF)
        in_maps.append(
            {
                "xt": xt_img,
                "wq": img_w(Wq[:, sl]),
                "wk": img_w(Wk[:, sl]),
                "wv": img_w(Wv[:, sl]),
                "wo": wo_img,
                "bqk": bqk_img,
                "bv": np.ascontiguousarray(bv[sl].reshape(1, DHC)),
            }
        )
    return in_maps


def kernel(x, Wq, bq, Wk, bk, Wv, bv, Wo, bo, _trace=False):
    from concourse.bass_utils import run_bass_kernel_spmd

    args = [np.asarray(a, dtype=np.float32) for a in (x, Wq, bq, Wk, bk, Wv, bv, Wo, bo)]
    if "nc" not in _cache:
        _cache["nc"] = _build()
    nc = _cache["nc"]

    in_maps = _pack_inputs(*args)
    res = run_bass_kernel_spmd(nc, in_maps, list(range(NCORE)), trace=_trace)
    _cache["last_result"] = res

    out = np.zeros((B, S, D), dtype=np.float32)
    for i in range(NCORE):
        out[i // TP] += res.results[i]["y"].astype(np.float32)
    out += np.asarray(args[8])  # bo, added once per (b, s) row on the host
    return out

